# revision 1
# baseline (speedup 1.0000x reference)
"""GAT layer kernel for Trainium2, 8 NeuronCores.

Sharding: 16 (b, h) pairs -> 8 cores. Core k handles batch b = k//2 and the
head pair hp = k%2 (heads 2*hp, 2*hp+1). adj (as an additive fp16 mask, host
pre-transposed) is replicated; each core runs the full N^2 attention for its
two heads, then the pair of cores for one batch AllReduces the partial output
of the head-mixing linear.

Math per (b, h), with softmax over the *i* axis (rows) of e[i, j]:
  h    = x[b] @ W[h]                         [N, F]
  f1_i = h_i . a1,  f2_j = h_j . a2
  v[j, i]  = f1_i + f2_j + M[j, i]           (M = 0 on edge, -150 masked)
  L        = max(v, 0.2*v)                   (= leaky, exp-monotone safe)
  Em[j, i] = exp(L)    ;  s_j = sum_i Em[j, i]   (ACT accum_out, fused)
  g[j, :]  = h[j, :] / s_j
  hpT[f, i] = sum_j g[j, f] * Em[j, i]       (PE, transposed-out layout)
  out = leaky(relu(hp) cat-heads @ Wl.T + bl)
"""

import sys

import numpy as np

sys.path.insert(0, "/opt/trn_rl_repo")

from concourse import bacc, bass, dve_ops, mybir, tile  # noqa: E402
from concourse.bass_utils import run_bass_kernel_spmd  # noqa: E402
from concourse.dve_spec import C0, C1, C2, Spec, Src0, Src1, relu  # noqa: E402

# Fused leaky-relu of a masked outer sum, one DVE pass at 1x:
#   out = leaky(in0 + s0 + in1) = s1*v + imm2*relu(v),  v = in0 + s0 + in1
# (in0 = broadcast f1 row, s0 = per-partition f2, in1 = additive adj mask).
_v = (Src0 + C0) + Src1
LEAKY_MASK_ANT = dve_ops.DveOp(
    "LEAKY_MASK_ANT",
    Spec(
        body=_v * C1 + relu(_v) * C2,
        reference=lambda in0, in1, s0, s1, imm2: (
            lambda v: (v * s1 + np.maximum(v, 0) * imm2).astype(np.float32)
        )(in0.astype(np.float32) + s0 + in1),
    ),
    subdim=False,
    uops_sha={"v3": "61445124be53cf8e", "v4": "fd84e7f03d2c00e0"},
)
if LEAKY_MASK_ANT.name not in dve_ops._SUB_OPCODE_FOR_NAME:
    dve_ops.OPS.append(LEAKY_MASK_ANT)
    dve_ops._SUB_OPCODE_FOR_NAME[LEAKY_MASK_ANT.name] = (
        dve_ops._CUSTOM_DVE_ROW_BASE + len(dve_ops.OPS) - 1)
    dve_ops.CUSTOM_DVE_SPECS[LEAKY_MASK_ANT.name] = LEAKY_MASK_ANT.spec

B, N, C, F, H = 4, 2048, 256, 64, 4
P = 128
NT = N // P  # 16 j-tiles / n-chunks
CT = C // P  # 2 contraction tiles over Cin
IC = 512  # i-chunk (matmul moving free dim / psum bank)
NIC = N // IC  # 4
ALPHA = 0.2
MASKV = 150.0  # additive mask magnitude; exp(0.2 * -150) ~ 1e-13
NCORES = 8

F32 = mybir.dt.float32
F16 = mybir.dt.float16
ADD = mybir.AluOpType.add
MULT = mybir.AluOpType.mult
MAX = mybir.AluOpType.max

_CACHE = {}


def _build_program(host_combine=False):
    nc = bacc.Bacc("TRN2", target_bir_lowering=False, debug=False,
                   num_devices=NCORES)

    xT = nc.dram_tensor("xT", [C, N], F32, kind="ExternalInput")
    madd = nc.dram_tensor("madd", [N, N], F16, kind="ExternalInput")
    w = nc.dram_tensor("w", [2, C, F], F32, kind="ExternalInput")
    a1c = nc.dram_tensor("a1c", [2, F], F32, kind="ExternalInput")
    a2c = nc.dram_tensor("a2c", [2, F], F32, kind="ExternalInput")
    wlT = nc.dram_tensor("wlT", [P, F], F16, kind="ExternalInput")
    blt = nc.dram_tensor("blt", [P, NT * F], F32, kind="ExternalInput")
    out = nc.dram_tensor("out", [N, F], F32, kind="ExternalOutput")

    cc_in = nc.dram_tensor("cc_in", [N, F], F32)
    cc_out = nc.dram_tensor("cc_out", [N, F], F32)

    with tile.TileContext(nc) as tc:
        with (
            tc.tile_pool(name="const", bufs=1) as const,
            tc.tile_pool(name="head", bufs=1) as head,
            tc.tile_pool(name="vm", bufs=3) as vm_pool,
            tc.tile_pool(name="em", bufs=3) as em_pool,
            tc.tile_pool(name="g", bufs=4) as g_pool,
            tc.tile_pool(name="psA", bufs=2, space="PSUM") as psA,
            tc.tile_pool(name="psB", bufs=1, space="PSUM") as psB,
        ):
            # ---- constant loads -------------------------------------------
            xT_sb = const.tile([P, CT, N], F32)
            for ct in range(CT):
                nc.sync.dma_start(xT_sb[:, ct, :], xT[ct * P:(ct + 1) * P, :])
            madd_sb = [const.tile([P, N], F16, tag=f"madd{j}",
                                  name=f"madd_sb{j}")
                       for j in range(NT)]
            for jt in range(NT):
                nc.sync.dma_start(madd_sb[jt][:],
                                  madd[jt * P:(jt + 1) * P, :])
            w_sb = const.tile([P, 2, CT, F], F32)
            for hl in range(2):
                for ct in range(CT):
                    nc.sync.dma_start(w_sb[:, hl, ct, :],
                                      w[hl, ct * P:(ct + 1) * P, :])
            a1_sb = const.tile([F, 2], F32)
            a2_sb = const.tile([F, 2], F32)
            for hl in range(2):
                nc.sync.dma_start(a1_sb[:, hl:hl + 1],
                                  a1c[hl:hl + 1, :].rearrange("a f -> f a"))
                nc.sync.dma_start(a2_sb[:, hl:hl + 1],
                                  a2c[hl:hl + 1, :].rearrange("a f -> f a"))
            wlT_sb = const.tile([P, F], F16)
            nc.sync.dma_start(wlT_sb[:], wlT[:])
            blt_sb = const.tile([P, NT * F], F32)
            nc.sync.dma_start(blt_sb[:], blt[:])
            ones_sb = const.tile([1, P], F32)
            nc.vector.memset(ones_sb[:], 1.0)

            catT_sb = const.tile([P, N], F16)

            for hl in range(2):
                # ---- phase A: projections -------------------------------
                h_sb = head.tile([P, NT, F], F16, tag="h")
                hT_sb = head.tile([F, N], F32, tag="hT")
                f1r_sb = head.tile([1, N], F32, tag="f1r")
                F1B_sb = head.tile([P, N], F16, tag="F1B")
                f2c_sb = head.tile([P, NT], F32, tag="f2c")
                sc_sb = head.tile([P, NT], F32, tag="sc")
                rc_sb = head.tile([P, NT], F32, tag="rc")

                # hT[f, n] = sum_c W[c, f] * xT[c, n]
                for icc in range(NIC):
                    ps = psA.tile([F, IC], F32, tag="psum_a")
                    for ct in range(CT):
                        nc.tensor.matmul(
                            ps[:], w_sb[:, hl, ct, :],
                            xT_sb[:, ct, icc * IC:(icc + 1) * IC],
                            start=(ct == 0), stop=(ct == CT - 1))
                    nc.any.tensor_copy(hT_sb[:, icc * IC:(icc + 1) * IC],
                                       ps[:])
                # h[n, f] = sum_c xT[c, n] * W[c, f]   (fp16 for pass-2 g)
                # 8 n-chunks packed per psum bank -> 2 big copies
                for grp in range(2):
                    ps = psA.tile([P, IC], F32, tag="psum_a")
                    for k in range(8):
                        nt = grp * 8 + k
                        for ct in range(CT):
                            nc.tensor.matmul(
                                ps[:, k * F:(k + 1) * F],
                                xT_sb[:, ct, nt * P:(nt + 1) * P],
                                w_sb[:, hl, ct, :],
                                start=(ct == 0), stop=(ct == CT - 1))
                    nc.any.tensor_copy(
                        h_sb[:, grp * 8:(grp + 1) * 8, :],
                        ps[:].rearrange("p (k f) -> p k f", f=F))
                # f1 row [1, N] = a1 . hT  ;  f2 col per tile = hT.T @ a2
                for icc in range(NIC):
                    ps = psA.tile([1, IC], F32, tag="psum_a")
                    nc.tensor.matmul(ps[:], a1_sb[:, hl:hl + 1],
                                     hT_sb[:, icc * IC:(icc + 1) * IC],
                                     start=True, stop=True)
                    nc.any.tensor_copy(f1r_sb[:, icc * IC:(icc + 1) * IC],
                                       ps[:])
                ps_f2 = psA.tile([P, NT], F32, tag="psum_a")
                for jt in range(NT):
                    nc.tensor.matmul(ps_f2[:, jt:jt + 1],
                                     hT_sb[:, jt * P:(jt + 1) * P],
                                     a2_sb[:, hl:hl + 1],
                                     start=True, stop=True)
                nc.any.tensor_copy(f2c_sb[:], ps_f2[:])
                # F1B = broadcast f1 row across partitions (ones outer-prod)
                for icc in range(NIC):
                    ps = psA.tile([P, IC], F32, tag="psum_a")
                    nc.tensor.matmul(ps[:], ones_sb[:],
                                     f1r_sb[:, icc * IC:(icc + 1) * IC],
                                     start=True, stop=True)
                    nc.any.tensor_copy(F1B_sb[:, icc * IC:(icc + 1) * IC],
                                       ps[:])

                # ---- hot loop: masked exp-leaky attention ---------------
                hpT = psB.tile([P, N], F32, tag="hpT")
                for jt in range(NT):
                    lk = vm_pool.tile([P, N], F16, tag="lk")
                    nc.vector._custom_dve(
                        LEAKY_MASK_ANT, out=lk[:], in0=F1B_sb[:],
                        in1=madd_sb[jt][:], s0=f2c_sb[:, jt:jt + 1],
                        s1=float(ALPHA), imm2=1.0 - ALPHA)
                    em = em_pool.tile([P, N], F16, tag="em")
                    nc.scalar.activation(
                        em[:], lk[:], mybir.ActivationFunctionType.Exp,
                        accum_out=sc_sb[:, jt:jt + 1])
                    nc.vector.reciprocal(rc_sb[:, jt:jt + 1],
                                         sc_sb[:, jt:jt + 1])
                    g = g_pool.tile([P, F], F16, tag="g")
                    nc.vector.tensor_scalar_mul(g[:], h_sb[:, jt, :],
                                                rc_sb[:, jt:jt + 1])
                    for icc in range(NIC):
                        nc.tensor.matmul(
                            hpT[hl * F:(hl + 1) * F,
                                icc * IC:(icc + 1) * IC],
                            g[:], em[:, icc * IC:(icc + 1) * IC],
                            start=(jt == 0), stop=(jt == NT - 1))
                # relu(hp) into the concat-head tile (same partitions)
                nc.scalar.activation(catT_sb[hl * F:(hl + 1) * F, :],
                                     hpT[hl * F:(hl + 1) * F, :],
                                     mybir.ActivationFunctionType.Relu)

            # ---- phase C: head-mixing linear + pair AllReduce -----------
            part_sb = const.tile([P, NT * F], F32)
            for grp in range(2):
                ps = psA.tile([P, IC], F32, tag="psum_a")
                for k in range(8):
                    ncu = grp * 8 + k
                    nc.tensor.matmul(ps[:, k * F:(k + 1) * F],
                                     catT_sb[:, ncu * P:(ncu + 1) * P],
                                     wlT_sb[:], start=True, stop=True)
                nc.any.tensor_copy(
                    part_sb[:, grp * IC:(grp + 1) * IC], ps[:])
            if host_combine:
                nc.sync.dma_start(
                    out.rearrange("(c p) f -> p c f", p=P),
                    part_sb[:].rearrange("p (c f) -> p c f", f=F))
            else:
                cc_in_v = cc_in.rearrange("(c p) f -> p c f", p=P)
                nc.sync.dma_start(cc_in_v, part_sb[:].rearrange(
                    "p (c f) -> p c f", f=F))
                nc.gpsimd.collective_compute(
                    "AllReduce", ADD,
                    replica_groups=[[0, 1], [2, 3], [4, 5], [6, 7]],
                    ins=[cc_in[:]], outs=[cc_out[:]])
                ys_sb = const.tile([P, NT * F], F32)
                nc.sync.dma_start(
                    ys_sb[:].rearrange("p (c f) -> p c f", f=F),
                    cc_out.rearrange("(c p) f -> p c f", p=P))
                yb_sb = const.tile([P, NT * F], F32)
                nc.vector.tensor_tensor(yb_sb[:], ys_sb[:], blt_sb[:],
                                        op=ADD)
                yo_sb = const.tile([P, NT * F], F32)
                nc.vector.scalar_tensor_tensor(
                    yo_sb[:], yb_sb[:], float(ALPHA), yb_sb[:],
                    op0=MULT, op1=MAX)
                nc.sync.dma_start(
                    out.rearrange("(c p) f -> p c f", p=P),
                    yo_sb[:].rearrange("p (c f) -> p c f", f=F))

    nc.compile()
    return nc


def get_program(host_combine=False):
    key = ("nc", host_combine)
    if key not in _CACHE:
        _CACHE[key] = _build_program(host_combine)
    return _CACHE[key]


def make_in_maps(x, adj, W, a1, a2, Wl, bl):
    x = np.asarray(x, dtype=np.float32)
    adj = np.asarray(adj)
    W = np.asarray(W, dtype=np.float32)
    a1 = np.asarray(a1, dtype=np.float32)
    a2 = np.asarray(a2, dtype=np.float32)
    Wl = np.asarray(Wl, dtype=np.float32)
    bl = np.asarray(bl, dtype=np.float32)

    madd = ((MASKV * adj.T.astype(np.float32)) - MASKV).astype(np.float16)
    madd = np.ascontiguousarray(madd)
    WlT = np.ascontiguousarray(Wl.T)  # [H*F, F]
    blt = np.ascontiguousarray(np.tile(bl, (P, NT)))

    in_maps = []
    for k in range(NCORES):
        b, hp = k // 2, k % 2
        hs = slice(2 * hp, 2 * hp + 2)
        in_maps.append({
            "xT": np.ascontiguousarray(x[b].T),
            "madd": madd,
            "w": np.ascontiguousarray(W[hs]),
            "a1c": np.ascontiguousarray(a1[hs]),
            "a2c": np.ascontiguousarray(a2[hs]),
            "wlT": np.ascontiguousarray(
                WlT[hp * P:(hp + 1) * P]).astype(np.float16),
            "blt": blt,
        })
    return in_maps


def kernel(x, adj, W, a1, a2, Wl, bl, _results=None, host_combine=False,
           **run_kwargs):
    nc = get_program(host_combine)
    in_maps = make_in_maps(x, adj, W, a1, a2, Wl, bl)
    res = run_bass_kernel_spmd(nc, in_maps, core_ids=list(range(NCORES)),
                               **run_kwargs)
    if _results is not None:
        _results.append(res)
    out = np.empty((B, N, F), dtype=np.float32)
    if host_combine:
        bl32 = np.asarray(bl, dtype=np.float32)
        for b in range(B):
            y = (res.results[2 * b]["out"] + res.results[2 * b + 1]["out"]
                 + bl32[None, :])
            out[b] = np.maximum(y, ALPHA * y)
    else:
        for b in range(B):
            out[b] = res.results[2 * b]["out"]
    return out



# revision 2
# speedup vs baseline: 1.6271x; 1.6271x over previous
"""GAT layer kernel for Trainium2, 8 NeuronCores.

Sharding: 16 (b, h) pairs -> 8 cores. Core k handles batch b = k//2 and the
head pair hp = k%2 (heads 2*hp, 2*hp+1). adj (as an additive fp16 mask, host
pre-transposed) is replicated; each core runs the full N^2 attention for its
two heads. The pair of cores for one batch combines the head-mixing linear
partials with a fp16 ReduceScatter (one per head, the first hidden under the
second head's hot loop); each core emits half the rows of the final output
and the host concatenates.

Math per (b, h), softmax over the *i* axis (rows) of e[i, j]:
  h    = x[b] @ W[h]                         [N, F]
  f1_i = h_i . a1,  f2_j = h_j . a2
  v[j, i]  = f1_i + f2_j + M[j, i]           (M = 0 on edge, -150 masked)
  L        = max(v, 0.2*v)                   (= leaky, exp-monotone safe)
  Em[j, i] = exp(L)    ;  s_j = sum_i Em[j, i]   (ACT accum_out, fused)
  g[j, :]  = h[j, :] / s_j
  hpT[f, i] = sum_j g[j, f] * Em[j, i]       (PE, transposed-out layout)
  out = leaky(relu(hp) cat-heads @ Wl.T + bl)

All projections run in fp16 on the PE (fp32 matmul double-pumps the array as
LOW/HIGH passes and was the top PE cost in the v1 trace). f1 is produced
directly in broadcast form (F1B) by a matmul against a host-replicated
W@a1 block, skipping the hT intermediate entirely.
"""

import sys

import numpy as np

sys.path.insert(0, "/opt/trn_rl_repo")

from concourse import bacc, bass, dve_ops, mybir, tile  # noqa: E402
from concourse.bass_utils import run_bass_kernel_spmd  # noqa: E402
from concourse.dve_spec import C0, C1, C2, Spec, Src0, Src1, relu  # noqa: E402

# Fused leaky-relu of a masked outer sum, one DVE pass at 1x:
#   out = leaky(in0 + s0 + in1) = s1*v + imm2*relu(v),  v = in0 + s0 + in1
# (in0 = broadcast f1 row, s0 = per-partition f2, in1 = additive adj mask).
_v = (Src0 + C0) + Src1
LEAKY_MASK_ANT = dve_ops.DveOp(
    "LEAKY_MASK_ANT",
    Spec(
        body=_v * C1 + relu(_v) * C2,
        reference=lambda in0, in1, s0, s1, imm2: (
            lambda v: (v * s1 + np.maximum(v, 0) * imm2).astype(np.float32)
        )(in0.astype(np.float32) + s0 + in1),
    ),
    subdim=False,
    uops_sha={"v3": "61445124be53cf8e", "v4": "fd84e7f03d2c00e0"},
)
if LEAKY_MASK_ANT.name not in dve_ops._SUB_OPCODE_FOR_NAME:
    dve_ops.OPS.append(LEAKY_MASK_ANT)
    dve_ops._SUB_OPCODE_FOR_NAME[LEAKY_MASK_ANT.name] = (
        dve_ops._CUSTOM_DVE_ROW_BASE + len(dve_ops.OPS) - 1)
    dve_ops.CUSTOM_DVE_SPECS[LEAKY_MASK_ANT.name] = LEAKY_MASK_ANT.spec

B, N, C, F, H = 4, 2048, 256, 64, 4
P = 128
NT = N // P  # 16 j-tiles / n-chunks
HNT = NT // 2  # 8 n-chunks per core in the final output half
CT = C // P  # 2 contraction tiles over Cin
IC = 512  # i-chunk (matmul moving free dim / psum bank)
NIC = N // IC  # 4
MG = 4  # madd DMA groups (of NT//MG j-tiles each)
ALPHA = 0.2
MASKV = 150.0  # additive mask magnitude; exp(0.2 * -150) ~ 1e-13
NCORES = 8

F32 = mybir.dt.float32
F16 = mybir.dt.float16
ADD = mybir.AluOpType.add
MULT = mybir.AluOpType.mult
MAX = mybir.AluOpType.max

_CACHE = {}


def _build_program():
    nc = bacc.Bacc("TRN2", target_bir_lowering=False, debug=False,
                   num_devices=NCORES)

    xT = nc.dram_tensor("xT", [C, N], F16, kind="ExternalInput")
    madd = nc.dram_tensor("madd", [N, N], F16, kind="ExternalInput")
    w = nc.dram_tensor("w", [2, C, F], F16, kind="ExternalInput")
    wa1r = nc.dram_tensor("wa1r", [2, C, P], F16, kind="ExternalInput")
    wa2 = nc.dram_tensor("wa2", [2, C, 1], F16, kind="ExternalInput")
    wlT = nc.dram_tensor("wlT", [P, F], F16, kind="ExternalInput")
    blt = nc.dram_tensor("blt", [P, HNT * F], F16, kind="ExternalInput")
    out = nc.dram_tensor("out", [N // 2, F], F32, kind="ExternalOutput")

    cc_in = [nc.dram_tensor(f"cc_in{hl}", [N, F], F16) for hl in range(2)]
    cc_out = [nc.dram_tensor(f"cc_out{hl}", [N // 2, F], F16)
              for hl in range(2)]

    with tile.TileContext(nc) as tc:
        with (
            tc.tile_pool(name="const", bufs=1) as const,
            tc.tile_pool(name="head", bufs=2) as head,
            tc.tile_pool(name="vm", bufs=3) as vm_pool,
            tc.tile_pool(name="em", bufs=3) as em_pool,
            tc.tile_pool(name="g", bufs=4) as g_pool,
            tc.tile_pool(name="psF", bufs=1, space="PSUM") as psF,
            tc.tile_pool(name="psH", bufs=1, space="PSUM") as psH,
            tc.tile_pool(name="psf2", bufs=1, space="PSUM") as psf2,
            tc.tile_pool(name="psFin", bufs=1, space="PSUM") as psFin,
            tc.tile_pool(name="psB", bufs=1, space="PSUM") as psB,
        ):
            # ---- constant loads -------------------------------------------
            xT_sb = const.tile([P, CT, N], F16)
            nc.sync.dma_start(xT_sb[:],
                              xT.rearrange("(c p) n -> p c n", p=P))
            w_sb = const.tile([P, 2, CT, F], F16)
            nc.sync.dma_start(w_sb[:],
                              w.rearrange("h (c p) f -> p h c f", p=P))
            wa1_sb = const.tile([P, 2, CT, P], F16)
            nc.sync.dma_start(wa1_sb[:],
                              wa1r.rearrange("h (c p) q -> p h c q", p=P))
            wa2_sb = const.tile([P, 2, CT, 1], F16)
            nc.sync.dma_start(wa2_sb[:],
                              wa2.rearrange("h (c p) o -> p h c o", p=P))
            wlT_sb = const.tile([P, F], F16)
            nc.sync.dma_start(wlT_sb[:], wlT[:])
            blt_sb = const.tile([P, HNT * F], F16)
            nc.sync.dma_start(blt_sb[:], blt[:])
            # madd in MG groups so the hot loop can start on group 0 early
            madd_sb = [const.tile([P, NT // MG, N], F16, tag=f"madd{g}",
                                  name=f"madd_sb{g}")
                       for g in range(MG)]
            for g in range(MG):
                rows = N // MG
                nc.sync.dma_start(
                    madd_sb[g][:],
                    madd[g * rows:(g + 1) * rows, :].rearrange(
                        "(t p) n -> p t n", p=P))

            catT_sb = const.tile([P, N], F16)
            part_sb = [const.tile([P, NT * F], F16, tag=f"part{hl}",
                                  name=f"part_sb{hl}") for hl in range(2)]

            for hl in range(2):
                # ---- phase A: fp16 projections ---------------------------
                F1B_sb = head.tile([P, N], F16, tag="F1B")
                h_sb = head.tile([P, NT, F], F16, tag="h")
                f2c_sb = head.tile([P, NT], F32, tag="f2c")
                sc_sb = head.tile([P, NT], F32, tag="sc")
                rc_sb = head.tile([P, NT], F32, tag="rc")

                # F1B[j, i] = f1[i] = sum_c wa1[c] * xT[c, i] (j-broadcast
                # via host-replicated wa1 columns)
                for icc in range(NIC):
                    ps = psF.tile([P, IC], F32, tag="psF")
                    for ct in range(CT):
                        nc.tensor.matmul(
                            ps[:], wa1_sb[:, hl, ct, :],
                            xT_sb[:, ct, icc * IC:(icc + 1) * IC],
                            start=(ct == 0), stop=(ct == CT - 1))
                    nc.any.tensor_copy(F1B_sb[:, icc * IC:(icc + 1) * IC],
                                       ps[:])
                # h[n, f] (fp16, for pass-2 g) and f2[n] share the xT
                # stationary; 8 n-chunks packed per psum bank
                for grp in range(2):
                    psh = psH.tile([P, IC], F32, tag="psH")
                    psf = psf2.tile([P, NT], F32, tag="psf2")
                    for k in range(8):
                        jt = grp * 8 + k
                        for ct in range(CT):
                            nc.tensor.matmul(
                                psh[:, k * F:(k + 1) * F],
                                xT_sb[:, ct, jt * P:(jt + 1) * P],
                                w_sb[:, hl, ct, :],
                                start=(ct == 0), stop=(ct == CT - 1))
                            nc.tensor.matmul(
                                psf[:, jt:jt + 1],
                                xT_sb[:, ct, jt * P:(jt + 1) * P],
                                wa2_sb[:, hl, ct, :],
                                start=(ct == 0), stop=(ct == CT - 1))
                    nc.any.tensor_copy(
                        h_sb[:, grp * 8:(grp + 1) * 8, :],
                        psh[:].rearrange("p (k f) -> p k f", f=F))
                    nc.any.tensor_copy(f2c_sb[:, grp * 8:(grp + 1) * 8],
                                       psf[:, grp * 8:(grp + 1) * 8])

                # ---- hot loop: masked exp-leaky attention ---------------
                hpT = psB.tile([P, N], F32, tag="hpT")
                for jt in range(NT):
                    mg, mk = jt // (NT // MG), jt % (NT // MG)
                    lk = vm_pool.tile([P, N], F16, tag="lk")
                    nc.vector._custom_dve(
                        LEAKY_MASK_ANT, out=lk[:], in0=F1B_sb[:],
                        in1=madd_sb[mg][:, mk, :],
                        s0=f2c_sb[:, jt:jt + 1],
                        s1=float(ALPHA), imm2=1.0 - ALPHA)
                    em = em_pool.tile([P, N], F16, tag="em")
                    nc.scalar.activation(
                        em[:], lk[:], mybir.ActivationFunctionType.Exp,
                        accum_out=sc_sb[:, jt:jt + 1])
                    nc.vector.reciprocal(rc_sb[:, jt:jt + 1],
                                         sc_sb[:, jt:jt + 1])
                    g = g_pool.tile([P, F], F16, tag="g")
                    nc.vector.tensor_scalar_mul(g[:], h_sb[:, jt, :],
                                                rc_sb[:, jt:jt + 1])
                    for icc in range(NIC):
                        nc.tensor.matmul(
                            hpT[hl * F:(hl + 1) * F,
                                icc * IC:(icc + 1) * IC],
                            g[:], em[:, icc * IC:(icc + 1) * IC],
                            start=(jt == 0), stop=(jt == NT - 1))
                # relu(hp) into the concat-head tile (same partitions)
                nc.scalar.activation(catT_sb[hl * F:(hl + 1) * F, :],
                                     hpT[hl * F:(hl + 1) * F, :],
                                     mybir.ActivationFunctionType.Relu)

                # ---- per-head final-linear partial + pair ReduceScatter.
                # Head 0's RS runs while head 1's hot loop computes.
                for grp in range(2):
                    ps = psFin.tile([P, IC], F32, tag="psFin")
                    for k in range(8):
                        ncu = grp * 8 + k
                        nc.tensor.matmul(
                            ps[:, k * F:(k + 1) * F],
                            catT_sb[hl * F:(hl + 1) * F,
                                    ncu * P:(ncu + 1) * P],
                            wlT_sb[hl * F:(hl + 1) * F, :],
                            start=True, stop=True)
                    nc.any.tensor_copy(
                        part_sb[hl][:, grp * IC:(grp + 1) * IC], ps[:])
                nc.sync.dma_start(
                    cc_in[hl].rearrange("(c p) f -> p c f", p=P),
                    part_sb[hl][:].rearrange("p (c f) -> p c f", f=F))
                nc.gpsimd.collective_compute(
                    "ReduceScatter", ADD,
                    replica_groups=[[0, 1], [2, 3], [4, 5], [6, 7]],
                    ins=[cc_in[hl][:]], outs=[cc_out[hl][:]])

            # ---- tail: combine the two head-pair halves -----------------
            ys_sb = [const.tile([P, HNT * F], F16, tag=f"ys{hl}",
                                name=f"ys_sb{hl}") for hl in range(2)]
            for hl in range(2):
                nc.sync.dma_start(
                    ys_sb[hl][:].rearrange("p (c f) -> p c f", f=F),
                    cc_out[hl].rearrange("(c p) f -> p c f", p=P))
            t1_sb = const.tile([P, HNT * F], F16)
            nc.vector.tensor_tensor(t1_sb[:], ys_sb[0][:], ys_sb[1][:],
                                    op=ADD)
            t2_sb = const.tile([P, HNT * F], F16)
            nc.vector.tensor_tensor(t2_sb[:], t1_sb[:], blt_sb[:], op=ADD)
            yo_sb = const.tile([P, HNT * F], F32)
            nc.vector.scalar_tensor_tensor(
                yo_sb[:], t2_sb[:], float(ALPHA), t2_sb[:],
                op0=MULT, op1=MAX)
            nc.sync.dma_start(
                out.rearrange("(c p) f -> p c f", p=P),
                yo_sb[:].rearrange("p (c f) -> p c f", f=F))

    nc.compile()
    return nc


def get_program():
    if "nc" not in _CACHE:
        _CACHE["nc"] = _build_program()
    return _CACHE["nc"]


def make_in_maps(x, adj, W, a1, a2, Wl, bl):
    x = np.asarray(x, dtype=np.float32)
    adj = np.asarray(adj)
    W = np.asarray(W, dtype=np.float32)
    a1 = np.asarray(a1, dtype=np.float32)
    a2 = np.asarray(a2, dtype=np.float32)
    Wl = np.asarray(Wl, dtype=np.float32)
    bl = np.asarray(bl, dtype=np.float32)

    madd = ((MASKV * adj.T.astype(np.float32)) - MASKV).astype(np.float16)
    madd = np.ascontiguousarray(madd)
    WlT = np.ascontiguousarray(Wl.T)  # [H*F, F]
    blt = np.tile(bl, (P, HNT)).astype(np.float16)
    wa1 = np.einsum("hcf,hf->hc", W, a1)  # [H, C]
    wa2 = np.einsum("hcf,hf->hc", W, a2)  # [H, C]

    in_maps = []
    for k in range(NCORES):
        b, hp = k // 2, k % 2
        hs = slice(2 * hp, 2 * hp + 2)
        wa1r = np.broadcast_to(wa1[hs][:, :, None], (2, C, P))
        in_maps.append({
            "xT": np.ascontiguousarray(x[b].T).astype(np.float16),
            "madd": madd,
            "w": W[hs].astype(np.float16),
            "wa1r": np.ascontiguousarray(wa1r).astype(np.float16),
            "wa2": wa2[hs][:, :, None].astype(np.float16),
            "wlT": np.ascontiguousarray(
                WlT[hp * P:(hp + 1) * P]).astype(np.float16),
            "blt": blt,
        })
    return in_maps


def assemble_out(per_core_out):
    """per_core_out[k] = [N//2, F] half owned by core k; returns [B, N, F]."""
    out = np.empty((B, N, F), dtype=np.float32)
    for b in range(B):
        out[b, :N // 2] = per_core_out[2 * b]
        out[b, N // 2:] = per_core_out[2 * b + 1]
    return out


def kernel(x, adj, W, a1, a2, Wl, bl, _results=None, **run_kwargs):
    nc = get_program()
    in_maps = make_in_maps(x, adj, W, a1, a2, Wl, bl)
    res = run_bass_kernel_spmd(nc, in_maps, core_ids=list(range(NCORES)),
                               **run_kwargs)
    if _results is not None:
        _results.append(res)
    return assemble_out([res.results[k]["out"] for k in range(NCORES)])
